# revision 7
# baseline (speedup 1.0000x reference)
"""Trainium2 Bass kernel for DiagonalVectorSpinGlassAttention.

Math (derived analytically from the reference; verified vs jax.jacrev): with
xs = per-head unit-normalized x, for each head h

    q = xs @ Wq_h^T          k = xs @ Wk_h^T          (n, 64)
    E = exp(q k^T)           r = rowsum(E)
    out[:, h] = (E @ k')/r + E^T @ (q'/r) + c0 * xs_h
      where k' = k @ Wq_hh, q' = q @ Wk_hh  (Wq_hh/Wk_hh = (64,64) diagonal
      blocks of W_qk for head h; projection folded in by associativity)

The kernel ships w^T = (E^T q'/r)^T and the UNnormalized u^T = (E @ k')^T
plus r; the host divides u by r, transposes, and adds the c0*xs term.

Sharding: 16 work units over 8 cores, uniform SPMD:
 - slot 0: one full head (heads 0..7 -> cores 0..7)
 - slot 1: one HALF of a head split over token rows (heads 8..11; core pair
   (2k, 2k+1) shares head 8+k). Odd cores get a half-swapped token order so
   every core owns "tiles 0..3"; the host adds the pair's partial w-terms.

Engine budget per core: PE ~52K cols (proj 12.3K fp8-input, sim 12.3K, w
12.3K, u 12.3K, k'/q' 1.8K, one PE-transposed tail tile), scalar = 12 exps
(+accum reads), vector = psum->sbuf copies + recips + q'r scaling, E^T via
XBAR dma-transposes whose ~2us completion latency hides mid-loop.
"""

import numpy as np
import ml_dtypes

import concourse.bass as bass
import concourse.tile as tile
from concourse import mybir
from concourse import bass_utils
from concourse.masks import make_identity

H, D = 12, 64
N = 1024
DIM = H * D  # 768
P = 128
NC = DIM // P  # 6 contraction tiles
NCORES = 8
SLOTS = 2
NTS = (8, 4)  # own token tiles per slot
C0 = np.float32(0.5 / ((0.5 + np.sqrt(1.25)) / 2.0))  # 0.618034
F32 = mybir.dt.float32
BF16 = mybir.dt.bfloat16
F8 = mybir.dt.float8e4
Exp = None  # set in body

_cache = {}


def _ts(i, size):
    return slice(i * size, (i + 1) * size)


def _build_kernel_body(tc):
    nc = tc.nc
    Exp = mybir.ActivationFunctionType.Exp

    at_d = nc.dram_tensor("at", (P, NC, N), F8, kind="ExternalInput").ap()
    wqk_d = nc.dram_tensor("wqk", (SLOTS, P, NC, P), F8,
                           kind="ExternalInput").ap()
    whq_d = nc.dram_tensor("whq", (SLOTS, D, D), BF16, kind="ExternalInput").ap()
    whk_d = nc.dram_tensor("whk", (SLOTS, D, D), BF16, kind="ExternalInput").ap()
    wt_d = nc.dram_tensor("wt", (P, N), BF16, kind="ExternalOutput").ap()
    ut_d = nc.dram_tensor("ut", (P, N), BF16, kind="ExternalOutput").ap()
    racc_d = nc.dram_tensor("racc", (P, 12), F32, kind="ExternalOutput").ap()

    import contextlib

    ctx = contextlib.ExitStack()
    with ctx:
        const = ctx.enter_context(tc.tile_pool(name="const", bufs=1))
        sb = ctx.enter_context(tc.tile_pool(name="sb", bufs=1))
        pp_sim = ctx.enter_context(tc.tile_pool(name="pp_sim", bufs=2,
                                                space="PSUM"))
        pp_w = ctx.enter_context(tc.tile_pool(name="pp_w", bufs=1, space="PSUM"))
        pp_sm = ctx.enter_context(tc.tile_pool(name="pp_sm", bufs=2,
                                               space="PSUM"))

        ident = const.tile([P, P], BF16)
        wu = const.tile([P, 512], BF16, name="wu")

        # ---- inputs first; every transfer has long contiguous rows
        at_sb = const.tile([P, NC, N], F8, name="at_sb")
        wqk_sb = [const.tile([P, NC, P], F8, tag=f"wqk{s}", name=f"wqk{s}")
                  for s in range(SLOTS)]
        whq_sb = [const.tile([D, D], BF16, tag=f"whq{s}", name=f"whq{s}")
                  for s in range(SLOTS)]
        whk_sb = [const.tile([D, D], BF16, tag=f"whk{s}", name=f"whk{s}")
                  for s in range(SLOTS)]
        nc.sync.dma_start(wqk_sb[0][:], wqk_d[0])
        nc.sync.dma_start(at_sb[:, 0:2, :], at_d[:, 0:2, :])
        nc.scalar.dma_start(at_sb[:, 2:4, :], at_d[:, 2:4, :])
        nc.gpsimd.dma_start(at_sb[:, 4:6, :], at_d[:, 4:6, :])
        nc.gpsimd.dma_start(wqk_sb[1][:], wqk_d[1])
        for s in range(SLOTS):
            nc.gpsimd.dma_start(whq_sb[s][:], whq_d[s])
            nc.gpsimd.dma_start(whk_sb[s][:], whk_d[s])

        nc.vector.memset(wu[:], 0.25)
        make_identity(nc, ident[:])
        # warm the scalar-engine exp table while input DMAs are in flight
        warm = const.tile([P, 1], F32)
        nc.scalar.activation(warm[:], ident[:, 0:1], Exp)
        # ramp the PE clock (HAM) with dummy matmuls while inputs land
        ps_wu = pp_sm.tile([P, 512], F32, tag="kq", name="ps_wu")
        for _ in range(5):
            nc.tensor.matmul(ps_wu[:], lhsT=wu[:, 0:P], rhs=wu[:],
                             start=True, stop=True)

        # ---- persistent sbuf state
        qT = [sb.tile([D, N], BF16, name=f"qT{s}") for s in range(SLOTS)]
        kT = [sb.tile([D, N], BF16, name=f"kT{s}") for s in range(SLOTS)]
        ktmp = sb.tile([P, N], BF16, name="ktmp")  # staging for slot1 k shift
        e1 = [sb.tile([P, NTS[s], N], BF16, name=f"e1{s}") for s in range(SLOTS)]
        e2 = [sb.tile([P, 8, NTS[s] * P], BF16, name=f"e2{s}")
              for s in range(SLOTS)]
        kp = [sb.tile([P, 8, D], BF16, name=f"kp{s}") for s in range(SLOTS)]
        qp = [sb.tile([P, NTS[s], D], BF16, name=f"qp{s}") for s in range(SLOTS)]
        qpr = [sb.tile([P, NTS[s], D], BF16, name=f"qpr{s}")
               for s in range(SLOTS)]
        racc = sb.tile([P, 12], F32, name="racc")
        recip = sb.tile([P, 12], F32, name="recip")
        wt_sb = sb.tile([P, N], BF16, name="wt_sb")
        ut_sb = sb.tile([P, N], BF16, name="ut_sb")

        ps_w = pp_w.tile([P, N], F32, name="ps_w")  # w^T both slots stacked

        rcol = [0, 8]  # racc column base per slot

        def proj(s):
            # c-outer so the first matmuls start as soon as at chunk 0 lands
            ps = pp_sim.tile([P, N], F32, tag="sim", name=f"ps_p{s}")
            for c in range(NC):
                for hf in range(2):
                    nc.tensor.matmul(
                        ps[:, _ts(hf, 512)],
                        lhsT=wqk_sb[s][:, c, :],
                        rhs=at_sb[:, c, _ts(hf, 512)],
                        start=(c == 0),
                        stop=(c == NC - 1),
                    )
            return ps

        sim_ps = {}

        def sim(s, t):
            ps = pp_sim.tile([P, N], F32, tag="sim", name=f"ps_s{s}{t}")
            for hf in range(2):
                nc.tensor.matmul(
                    ps[:, _ts(hf, 512)],
                    lhsT=qT[s][:, _ts(t, P)],
                    rhs=kT[s][:, _ts(hf, 512)],
                    start=True,
                    stop=True,
                )
            sim_ps[(s, t)] = ps

        def expt(s, t):
            c = rcol[s] + t
            nc.scalar.activation(
                e1[s][:, t, :], sim_ps.pop((s, t))[:], Exp,
                accum_out=racc[:, c : c + 1],
            )

        def qprep(s):
            # q' = q @ Wk_hh for all own tiles (no recip yet -> early psum free)
            ps = pp_sm.tile([P, 8, D], F32, tag="kq", name=f"ps_q{s}")
            for t in range(NTS[s]):
                nc.tensor.matmul(
                    ps[:, t, :], lhsT=qT[s][:, _ts(t, P)], rhs=whk_sb[s][:],
                    start=True, stop=True,
                )
            nc.vector.tensor_copy(qp[s][:], ps[:, 0 : NTS[s], :])

        def kprep(s):
            # k'_tok = k @ Wq_hh, token-major (j on partitions)
            ps = pp_sm.tile([P, 8, D], F32, tag="kq", name=f"ps_k{s}")
            for jt in range(8):
                nc.tensor.matmul(
                    ps[:, jt, :], lhsT=kT[s][:, _ts(jt, P)], rhs=whq_sb[s][:],
                    start=True, stop=True,
                )
            nc.vector.tensor_copy(kp[s][:], ps[:])

        def post(s, t):
            # recip + q'r scaling (vector) after exp(s, t)
            c = rcol[s] + t
            nc.vector.reciprocal(recip[:, c : c + 1], racc[:, c : c + 1])
            nc.vector.tensor_scalar_mul(
                qpr[s][:, t, :], qp[s][:, t, :], recip[:, c : c + 1]
            )

        def wch(s, t):
            # w^T[d', j] += q'r[own i tile t] . E1[t]  (contraction over i)
            for hf in range(2):
                nc.tensor.matmul(
                    ps_w[_ts(s, D), _ts(hf, 512)],
                    lhsT=qpr[s][:, t, :],
                    rhs=e1[s][:, t, _ts(hf, 512)],
                    start=(t == 0),
                    stop=(t == NTS[s] - 1),
                )

        _dmaq = [0]

        def dmaT(s, t):
            # all on sync: the 1.3us descriptor-gen must stay OFF scalar
            nc.sync.dma_start_transpose(e2[s][:, :, _ts(t, P)], e1[s][:, t, :])

        def uchunk(s, cols, tag):
            # u^T[d', cols] = sum_jt k'[jt] . E2[jt, cols]
            ps = pp_sm.tile([P, 512], F32, tag="kq", name=f"ps_u{tag}")
            w = cols.stop - cols.start
            for jt in range(8):
                nc.tensor.matmul(
                    ps[0:D, 0:w],
                    lhsT=kp[s][:, jt, :],
                    rhs=e2[s][:, jt, cols],
                    start=(jt == 0),
                    stop=(jt == 7),
                )
            nc.vector.tensor_copy(ut_sb[_ts(s, D), cols], ps[0:D, 0:w])

        # ---------------- emission schedule ----------------
        # Cross-engine syncs are program-order counters: a consumer waits for
        # ALL producer-engine work emitted before it. So each exp is emitted
        # IMMEDIATELY after its sim; all other PE work goes after the exp.
        ps_p0 = proj(0)
        # psum -> sbuf: q on vector, k on scalar (scalar idle pre-exp);
        # slot1 k goes vector + partition-shifting sb2sb DMA to spare scalar
        nc.vector.tensor_copy(qT[0][:], ps_p0[0:D, :])
        nc.scalar.copy(kT[0][:], ps_p0[D:P, :])
        ps_p1 = proj(1)
        sim(0, 0)
        expt(0, 0)
        nc.vector.tensor_copy(ktmp[D:P, :], ps_p1[D:P, :])
        nc.gpsimd.dma_start(kT[1][:], ktmp[D:P, :])
        nc.vector.tensor_copy(qT[1][:], ps_p1[0:D, :])
        sim(0, 1)
        expt(0, 1)
        qprep(0)
        kprep(0)
        post(0, 0)
        # steady state: sim(t) ; exp(t) ; w(t-2) ; dmaT(t-2) ; post(t-1)
        for t in range(2, 8):
            sim(0, t)
            expt(0, t)
            wch(0, t - 2)
            dmaT(0, t - 2)
            post(0, t - 1)
            if t == 7:
                qprep(1)
                kprep(1)
        sim(1, 0)
        expt(1, 0)
        wch(0, 6)
        dmaT(0, 6)
        post(0, 7)
        sim(1, 1)
        expt(1, 1)
        wch(0, 7)
        dmaT(0, 7)
        post(1, 0)
        # w slot0 done: copy rows 0:64 out early (overlaps slot1 exps)
        nc.vector.tensor_copy(wt_sb[0:D, :], ps_w[0:D, :])
        nc.gpsimd.dma_start(wt_d[0:D, :], wt_sb[0:D, :])
        sim(1, 2)
        expt(1, 2)
        wch(1, 0)
        dmaT(1, 0)
        post(1, 1)
        uchunk(0, slice(0, 512), "0a")  # needs dmaT(0,0..3): long landed
        sim(1, 3)
        expt(1, 3)
        wch(1, 1)
        dmaT(1, 1)
        post(1, 2)
        wch(1, 2)
        dmaT(1, 2)
        post(1, 3)
        uchunk(0, slice(512, 1024), "0b")  # needs dmaT(0,4..7)
        nc.gpsimd.dma_start(ut_d[0:D, :], ut_sb[0:D, :])
        # tail tile (1,3): PE transpose (dmaT latency too long here)
        for g in range(2):
            tp = pp_sm.tile([P, 4, P], BF16, tag="kq", name=f"tp{g}")
            for k in range(4):
                nc.tensor.transpose(
                    tp[:, k, :], e1[1][:, 3, _ts(4 * g + k, P)], ident[:]
                )
            nc.vector.tensor_copy(
                e2[1][:, _ts2(4 * g, 4), 384:512], tp[:]
            )
        wch(1, 3)
        # w slot1 copy on scalar (vector busy with u copies)
        nc.scalar.copy(wt_sb[D:P, :], ps_w[D:P, :])
        nc.gpsimd.dma_start(wt_d[D:P, :], wt_sb[D:P, :])
        uchunk(1, slice(0, 384), "1a")  # dmaT(1,0..2) landed
        uchunk(1, slice(384, 512), "1b")
        nc.gpsimd.dma_start(ut_d[D:P, 0:512], ut_sb[D:P, 0:512])
        nc.gpsimd.dma_start(racc_d, racc[:])


def _ts2(i, m):
    return slice(i, i + m)


def _split_multi_waits(nc, limit=1):
    """The walrus build in this container encodes at most one sync-wait per
    instruction. Move extra waits onto NoOp carrier instructions inserted
    just before the offending instruction on the same engine."""
    n_nop = 0
    for fn in nc.m.functions:
        for blk in fn.blocks:
            il = blk.instructions
            idx = 0
            while idx < len(il):
                inst = il[idx]
                si = inst.sync_info
                if si is not None and len(si.on_wait) > limit:
                    waits = list(si.on_wait)
                    extra, keep = waits[:-limit], waits[-limit:]
                    inst.sync_info = mybir.SyncInfo(
                        on_wait=keep, on_update=list(si.on_update)
                    )
                    for w in extra:
                        nop = mybir.InstNoOp(name=f"waitnop-{n_nop}", ins=[],
                                             outs=[])
                        n_nop += 1
                        nop.engine = inst.engine
                        nop.sync_info = mybir.SyncInfo(on_wait=[w], on_update=[])
                        il.insert(idx, nop)
                        idx += 1
                idx += 1
    return n_nop


def _get_nc(split_waits=True):
    key = ("nc", split_waits)
    if key not in _cache:
        nc = bass.Bass("TRN2", debug=False, target_bir_lowering=False,
                       num_devices=NCORES)
        with tile.TileContext(nc) as tc:
            _build_kernel_body(tc)
        if split_waits:
            _split_multi_waits(nc)
        _cache[key] = nc
    return _cache[key]


def _prep_inputs(x, W_qk):
    x = np.asarray(x, dtype=np.float32)
    W = np.asarray(W_qk, dtype=np.float32)
    n = x.shape[0]
    xh = x.reshape(n, H, D)
    nrm = np.sqrt(np.sum(xh * xh, axis=-1, keepdims=True, dtype=np.float32))
    xh = (xh / nrm).astype(np.float32)
    A = np.ascontiguousarray(xh.reshape(n, DIM))

    swap = np.concatenate([np.arange(N // 2, N), np.arange(N // 2)])
    ident_perm = np.arange(N)
    f8 = ml_dtypes.float8_e4m3

    def pack_at(A_perm):
        # at[p, c, tok] = A_perm[tok, c*128+p]
        return np.ascontiguousarray(
            A_perm.T.reshape(NC, P, N).transpose(1, 0, 2)
        ).astype(f8)

    at_by_perm = {0: pack_at(A), 1: pack_at(A[swap])}

    in_maps = []
    perms = []
    for c in range(NCORES):
        half = c % 2
        perm = ident_perm if half == 0 else swap
        perms.append(perm)
        heads = [c, 8 + c // 2]
        wqk = np.zeros((SLOTS, P, NC, P), dtype=f8)
        whq = np.zeros((SLOTS, D, D), dtype=ml_dtypes.bfloat16)
        whk = np.zeros((SLOTS, D, D), dtype=ml_dtypes.bfloat16)
        for s in range(SLOTS):
            h = heads[s]
            Wq_h = W[h * D : (h + 1) * D, :]              # (64, 768)
            Wk_h = W[DIM + h * D : DIM + (h + 1) * D, :]
            Wrow = np.vstack([Wq_h, Wk_h])                # (128, 768)
            wqk[s] = Wrow.T.reshape(NC, P, P).transpose(1, 0, 2).astype(f8)
            whq[s] = Wq_h[:, h * D : (h + 1) * D].astype(ml_dtypes.bfloat16)
            whk[s] = Wk_h[:, h * D : (h + 1) * D].astype(ml_dtypes.bfloat16)
        in_maps.append({
            "at": at_by_perm[half],
            "wqk": np.ascontiguousarray(wqk),
            "whq": np.ascontiguousarray(whq),
            "whk": np.ascontiguousarray(whk),
        })
    return in_maps, A, perms


def kernel(x, mask, W_qk, trace=False):
    nc = _get_nc()
    in_maps, A, perms = _prep_inputs(x, W_qk)
    res = bass_utils.run_bass_kernel_spmd(
        nc, in_maps, core_ids=list(range(NCORES)), trace=trace
    )
    _cache["last_results"] = res

    out = np.empty((N, DIM), dtype=np.float32)
    acc = {}
    for c in range(NCORES):
        perm = perms[c]
        r = res.results[c]
        wt = np.asarray(r["wt"]).astype(np.float32)    # (128, 1024)
        ut = np.asarray(r["ut"]).astype(np.float32)
        rc = np.asarray(r["racc"]).astype(np.float32)  # (128, 12)
        # slot 0: full head c
        rvec = rc[:, 0:8].T.reshape(-1)                # token-major (perm space)
        blk = ut[0:D, :].T / rvec[:, None] + wt[0:D, :].T
        out[perm, c * D : (c + 1) * D] = blk
        # slot 1: half of head 8 + c//2
        g = 8 + c // 2
        rvec1 = rc[:, 8:12].T.reshape(-1)              # own 512 tokens
        a = acc.setdefault(g, np.zeros((N, D), dtype=np.float32))
        a[perm] += wt[D:P, :].T
        a[perm[:512]] += ut[D:P, 0:512].T / rvec1[:, None]
    for g, a in acc.items():
        out[:, g * D : (g + 1) * D] = a
    out += C0 * A
    return out
